# revision 1
# baseline (speedup 1.0000x reference)
"""Trainium2 Bass kernel for nn_Deepset GNN message-passing problem.

Computation:
    h  = relu(x @ W1 + b1)          # [N, 64]   (x: [400000, 1024])
    h2 = h @ W2 + b2                # [N, 64]
    pooled = segment_mean(h2, batch, 512)
    z = (pooled @ W3 + b3) @ W4 + b4
    out = softmax(z, axis=0)        # [512, 2]

Device does the dominant work: h = relu(x@W1+b1) and a per-graph
segment-sum of h. Everything downstream of the [512, 64] segment sums
(~2 MFLOP) runs on host.

Sharding: data-parallel over nodes, 50000 nodes/core on 8 cores.
`batch` is sorted, so each core's graph ids span < 128 consecutive
values; ids are shifted into a per-core [0, 128) window on host and the
per-core partial sums S_i [128, 64] are overlap-added on host.

Device pipeline per core (bf16 compute, fp32 accumulation):
  - x shard is cast to bf16 and packed tile-major on host so each
    512-node tile is one fully contiguous [128 partitions x 8KB] DMA
  - per 512-node tile: hT[64,512] = sum_k W1_chunk.T @ xT_chunk (PE),
    relu+bias+bf16 cast (ScalarE, PSUM->SBUF),
    PE-transpose of 128-node slices to natural [128, 64] layout,
    one-hot(batch) built on DVE via iota==graph_id per-partition compare,
    segment-sum as one-hot matmul accumulating into one PSUM bank
    across the whole kernel.
"""

import numpy as np

N_NODES = 400000
D_FEAT = 1024
HIDDEN = 64
NUM_GRAPHS = 512
N_CORES = 8
NPC = N_NODES // N_CORES        # 50000 nodes per core
TILE_N = 512                    # nodes per PE tile
N_PAD = 50176                   # 98 * 512
N_TILES = N_PAD // TILE_N       # 98
CHUNK = 128                     # nodes per segment-sum chunk
KC = D_FEAT // 128              # 8 contraction chunks
GWIN = 128                      # per-core graph-id window

LAST_RESULT = None              # BassKernelResults of the last run (for profiling)


def _build_nc(d_feat=D_FEAT, n_pad=N_PAD, tile_n=TILE_N, hidden=HIDDEN,
              chunk=CHUNK, gwin=GWIN, repeat=1, xp_bufs=16, dma_split=2):
    import concourse.bass as bass
    import concourse.bacc as bacc
    import concourse.tile as tile
    from concourse import mybir
    from contextlib import ExitStack

    dt = mybir.dt
    kc = d_feat // 128
    n_tiles = n_pad // tile_n
    n_chunks = n_pad // chunk
    cpt = tile_n // chunk       # chunks per tile

    nc = bacc.Bacc("TRN2", target_bir_lowering=False, debug=False)
    xT = nc.declare_dram_parameter("xT", [n_pad // tile_n, 128, (d_feat // 128) * tile_n],
                               dt.bfloat16, isOutput=False)
    w1 = nc.declare_dram_parameter("w1", [d_feat, hidden], dt.bfloat16, isOutput=False)
    b1 = nc.declare_dram_parameter("b1", [hidden, 1], dt.float32, isOutput=False)
    bsh = nc.declare_dram_parameter("bsh", [chunk, n_chunks], dt.float32, isOutput=False)
    iota = nc.declare_dram_parameter("iota", [chunk, gwin], dt.bfloat16, isOutput=False)
    ident = nc.declare_dram_parameter("ident", [hidden, hidden], dt.bfloat16, isOutput=False)
    sout = nc.declare_dram_parameter("sout", [gwin, hidden], dt.float32, isOutput=True)

    w1_r = w1[:, :].rearrange("(c p) h -> p c h", p=128)

    with ExitStack() as ctx:
        tc = ctx.enter_context(tile.TileContext(nc))
        const = ctx.enter_context(tc.tile_pool(name="const", bufs=1))
        xp = ctx.enter_context(tc.tile_pool(name="xp", bufs=xp_bufs))
        htp = ctx.enter_context(tc.tile_pool(name="htp", bufs=3, space=bass.MemorySpace.PSUM))
        hts = ctx.enter_context(tc.tile_pool(name="hts", bufs=3))
        hnp = ctx.enter_context(tc.tile_pool(name="hnp", bufs=3, space=bass.MemorySpace.PSUM))
        hns = ctx.enter_context(tc.tile_pool(name="hns", bufs=6))
        ohp = ctx.enter_context(tc.tile_pool(name="ohp", bufs=6))
        ssp = ctx.enter_context(tc.tile_pool(name="ssp", bufs=1, space=bass.MemorySpace.PSUM))

        w1_sb = const.tile([128, kc, hidden], dt.bfloat16)
        nc.sync.dma_start(w1_sb[:], w1_r)
        b1_sb = const.tile([hidden, 1], dt.float32)
        nc.sync.dma_start(b1_sb[:], b1[:, :])
        bsh_sb = const.tile([chunk, n_chunks], dt.float32)
        nc.sync.dma_start(bsh_sb[:], bsh[:, :])
        iota_sb = const.tile([chunk, gwin], dt.bfloat16)
        nc.sync.dma_start(iota_sb[:], iota[:, :])
        ident_sb = const.tile([hidden, hidden], dt.bfloat16)
        nc.sync.dma_start(ident_sb[:], ident[:, :])

        # Per-graph sums accumulate in a single PSUM bank across all chunks
        # of the kernel (start only on the very first chunk).
        s_ps = ssp.tile([gwin, hidden], dt.float32)

        # Segment-sum tail for one tile: PE-transpose each 128-node slice to
        # natural layout, build the one-hot on DVE, accumulate via matmul.
        def seg_tail(r, t, ht_sb):
            for c4 in range(cpt):
                c = cpt * t + c4
                h_ps = hnp.tile([chunk, hidden], dt.bfloat16)
                nc.tensor.transpose(h_ps[:], ht_sb[:, c4 * chunk:(c4 + 1) * chunk],
                                    ident_sb[:])
                h_sb = hns.tile([chunk, hidden], dt.bfloat16)
                nc.vector.tensor_copy(h_sb[:], h_ps[:])
                oh = ohp.tile([chunk, gwin], dt.bfloat16)
                nc.vector.tensor_single_scalar(oh[:], iota_sb[:], bsh_sb[:, c:c + 1],
                                               mybir.AluOpType.is_equal)
                nc.tensor.matmul(s_ps[:], oh[:], h_sb[:],
                                 start=(r == 0 and c == 0),
                                 stop=(r == repeat - 1 and c == n_chunks - 1),
                                 skip_group_check=True)

        # PE is in-order, so tile t's transposes (gated on the ScalarE relu)
        # are emitted after tile t+1's main matmuls — the relu latency hides
        # under them instead of stalling the PE stream.
        pending = None
        for r in range(repeat):  # repeat>1 is a bench-only mode
            for t in range(n_tiles):
                xt = xp.tile([128, kc, tile_n], dt.bfloat16)
                xsrc = xT[t, :, :].rearrange("p (c n) -> p c n", c=kc)
                ks = kc // dma_split
                for s in range(dma_split):
                    nc.sync.dma_start(xt[:, s * ks:(s + 1) * ks, :],
                                      xsrc[:, s * ks:(s + 1) * ks, :])

                ht_ps = htp.tile([hidden, tile_n], dt.float32)
                for k in range(kc):
                    nc.tensor.matmul(ht_ps[:], w1_sb[:, k, :], xt[:, k, :],
                                     start=(k == 0), stop=(k == kc - 1))

                if pending is not None:
                    seg_tail(*pending)

                ht_sb = hts.tile([hidden, tile_n], dt.bfloat16)
                nc.scalar.activation(ht_sb[:], ht_ps[:],
                                     mybir.ActivationFunctionType.Relu, bias=b1_sb[:])
                pending = (r, t, ht_sb)
        seg_tail(*pending)

        s_sb = const.tile([gwin, hidden], dt.float32)
        nc.vector.tensor_copy(s_sb[:], s_ps[:])
        nc.sync.dma_start(sout[:, :], s_sb[:])

    nc.compile()
    return nc


def _prep_inputs(x, batch):
    """Per-core input maps + per-core graph-window bases."""
    import ml_dtypes
    bf16 = np.dtype(ml_dtypes.bfloat16)

    iota_np = np.ascontiguousarray(
        np.broadcast_to(np.arange(GWIN, dtype=np.float32), (CHUNK, GWIN))).astype(bf16)
    ident_np = np.eye(HIDDEN, dtype=np.float32).astype(bf16)

    in_maps = []
    g_bases = []
    n_chunks = N_PAD // CHUNK
    for i in range(N_CORES):
        lo, hi = i * NPC, (i + 1) * NPC
        xs = np.zeros((N_PAD, D_FEAT), dtype=bf16)
        xs[:NPC] = x[lo:hi].astype(bf16)
        # tile-major pack: xTt[t, p, c*TILE_N + n] = x[t*TILE_N + n, c*128 + p]
        # so each 512-node tile is one fully-contiguous [128, 8KB] DMA.
        xT = np.ascontiguousarray(
            xs.reshape(N_TILES, TILE_N, KC, 128).transpose(0, 3, 2, 1)
        ).reshape(N_TILES, 128, KC * TILE_N)

        b = np.asarray(batch[lo:hi], dtype=np.int64)
        g0 = int(b[0])
        span = int(b[-1]) - g0
        assert span < GWIN, f"core {i}: graph span {span} >= {GWIN}"
        g_bases.append(g0)
        bshift = np.full((N_PAD,), -1.0, np.float32)
        bshift[:NPC] = (b - g0).astype(np.float32)
        bsh_np = np.ascontiguousarray(bshift.reshape(n_chunks, CHUNK).T)

        in_maps.append({
            "xT": xT,
            "bsh": bsh_np,
            "iota": iota_np,
            "ident": ident_np,
        })
    return in_maps, g_bases


def kernel(x, batch, W1, b1, W2, b2, W3, b3, W4, b4):
    global LAST_RESULT
    import ml_dtypes
    from concourse.bass_utils import run_bass_kernel_spmd

    bf16 = np.dtype(ml_dtypes.bfloat16)
    x = np.asarray(x)
    batch = np.asarray(batch)
    W1 = np.asarray(W1, np.float32)
    b1 = np.asarray(b1, np.float32)

    in_maps, g_bases = _prep_inputs(x, batch)
    w1_np = W1.astype(bf16)
    b1_np = b1.reshape(HIDDEN, 1).copy()
    for m in in_maps:
        m["w1"] = w1_np
        m["b1"] = b1_np

    nc = _build_nc()
    res = run_bass_kernel_spmd(nc, in_maps, list(range(N_CORES)))
    LAST_RESULT = res

    # Host-side: overlap-add per-core partial segment sums, then the tiny head.
    S = np.zeros((NUM_GRAPHS + GWIN, HIDDEN), np.float64)
    for i in range(N_CORES):
        g0 = g_bases[i]
        S[g0:g0 + GWIN] += np.asarray(res.results[i]["sout"], np.float64)
    S = S[:NUM_GRAPHS]

    cnt = np.bincount(batch.astype(np.int64), minlength=NUM_GRAPHS).astype(np.float64)
    meanh = S / np.maximum(cnt, 1.0)[:, None]
    pooled = meanh @ np.asarray(W2, np.float64) + np.asarray(b2, np.float64)
    pooled *= (cnt > 0)[:, None]  # empty graphs pool to exactly zero in the reference
    z = pooled @ np.asarray(W3, np.float64) + np.asarray(b3, np.float64)
    z = z @ np.asarray(W4, np.float64) + np.asarray(b4, np.float64)
    z -= z.max(axis=0, keepdims=True)
    e = np.exp(z)
    out = e / e.sum(axis=0, keepdims=True)
    return out.astype(np.float32)



# revision 7
# speedup vs baseline: 1.2302x; 1.2302x over previous
"""Trainium2 Bass kernel for nn_Deepset GNN message-passing problem.

Computation:
    h  = relu(x @ W1 + b1)          # [N, 64]   (x: [400000, 1024])
    h2 = h @ W2 + b2                # [N, 64]
    pooled = segment_mean(h2, batch, 512)
    z = (pooled @ W3 + b3) @ W4 + b4
    out = softmax(z, axis=0)        # [512, 2]

Device does the dominant work: h = relu(x@W1+b1) and a per-graph
segment-sum of h. Everything downstream of the [512, 64] segment sums
(~2 MFLOP) runs on host.

Sharding: data-parallel over nodes, 50000 nodes/core on 8 cores.
`batch` is sorted, so each core's graph ids span < 128 consecutive
values; ids are shifted into a per-core [0, 128) window on host and the
per-core partial sums S_i [128, 64] are overlap-added on host.

Device pipeline per core (fp8 DoubleRow compute, fp32 accumulation):
  - x shard is cast to fp8e4m3 and packed tile-major on host so each
    512-node tile is one fully contiguous [128 partitions x 4KB] DMA
  - W1 is pre-scaled by 64 into fp8e4m3 range; the relu activation
    un-scales with scale=1/64 (out = relu(psum/64 + b1))
  - per 512-node tile: hT[64,512] = sum of 4 DoubleRow matmuls, each
    contracting 256 features (lhsT [128,2,64], rhs [128,2,512]),
    relu+bias+bf16 cast (ScalarE, PSUM->SBUF),
    PE-transpose of 128-node slices to natural [128, 64] layout,
    one-hot(batch) built on DVE via iota==graph_id per-partition compare,
    segment-sum as one-hot matmul accumulating into one PSUM bank
    across the whole kernel.
"""

import numpy as np

N_NODES = 400000
D_FEAT = 1024
HIDDEN = 64
NUM_GRAPHS = 512
N_CORES = 8
NPC = N_NODES // N_CORES        # 50000 nodes per core
TILE_N = 512                    # nodes per PE tile
N_PAD = 50176                   # 98 * 512
N_TILES = N_PAD // TILE_N       # 98
CHUNK = 128                     # nodes per segment-sum chunk
KC = D_FEAT // 128              # 8 contraction chunks
GWIN = 128                      # per-core graph-id window

LAST_RESULT = None              # BassKernelResults of the last run (for profiling)


def _build_nc(d_feat=D_FEAT, n_pad=N_PAD, tile_n=TILE_N, hidden=HIDDEN,
              chunk=CHUNK, gwin=GWIN, repeat=1, xp_bufs=16, dma_split=2):
    import concourse.bass as bass
    import concourse.bacc as bacc
    import concourse.tile as tile
    from concourse import mybir
    from contextlib import ExitStack

    dt = mybir.dt
    kc = d_feat // 128
    n_tiles = n_pad // tile_n
    n_chunks = n_pad // chunk
    cpt = tile_n // chunk       # chunks per tile

    nc = bacc.Bacc("TRN2", target_bir_lowering=False, debug=False)
    xT = nc.declare_dram_parameter("xT", [n_pad // tile_n, 128, (d_feat // 128) * tile_n],
                               dt.float8e4, isOutput=False)
    w1 = nc.declare_dram_parameter("w1", [d_feat, hidden], dt.float8e4, isOutput=False)
    b1 = nc.declare_dram_parameter("b1", [hidden, 1], dt.float32, isOutput=False)
    bsh = nc.declare_dram_parameter("bsh", [chunk, n_chunks], dt.float32, isOutput=False)
    iota = nc.declare_dram_parameter("iota", [chunk, gwin], dt.bfloat16, isOutput=False)
    ident = nc.declare_dram_parameter("ident", [hidden, hidden], dt.bfloat16, isOutput=False)
    sout = nc.declare_dram_parameter("sout", [gwin, hidden], dt.float32, isOutput=True)

    w1_r = w1[:, :].rearrange("(c p) h -> p c h", p=128)

    with ExitStack() as ctx:
        tc = ctx.enter_context(tile.TileContext(nc))
        const = ctx.enter_context(tc.tile_pool(name="const", bufs=1))
        xp = ctx.enter_context(tc.tile_pool(name="xp", bufs=xp_bufs))
        htp = ctx.enter_context(tc.tile_pool(name="htp", bufs=3, space=bass.MemorySpace.PSUM))
        hts = ctx.enter_context(tc.tile_pool(name="hts", bufs=3))
        hnp = ctx.enter_context(tc.tile_pool(name="hnp", bufs=3, space=bass.MemorySpace.PSUM))
        hns = ctx.enter_context(tc.tile_pool(name="hns", bufs=6))
        ohp = ctx.enter_context(tc.tile_pool(name="ohp", bufs=6))
        ssp = ctx.enter_context(tc.tile_pool(name="ssp", bufs=1, space=bass.MemorySpace.PSUM))

        w1_sb = const.tile([128, kc, hidden], dt.float8e4)
        nc.sync.dma_start(w1_sb[:], w1_r)
        b1_sb = const.tile([hidden, 1], dt.float32)
        nc.sync.dma_start(b1_sb[:], b1[:, :])
        bsh_sb = const.tile([chunk, n_chunks], dt.float32)
        nc.sync.dma_start(bsh_sb[:], bsh[:, :])
        iota_sb = const.tile([chunk, gwin], dt.bfloat16)
        nc.sync.dma_start(iota_sb[:], iota[:, :])
        ident_sb = const.tile([hidden, hidden], dt.bfloat16)
        nc.sync.dma_start(ident_sb[:], ident[:, :])

        # Per-graph sums accumulate in a single PSUM bank across all chunks
        # of the kernel (start only on the very first chunk).
        s_ps = ssp.tile([gwin, hidden], dt.float32)

        # Segment-sum tail for one tile: PE-transpose each 128-node slice to
        # natural layout, build the one-hot on DVE, accumulate via matmul.
        def seg_tail(r, t, ht_sb):
            for c4 in range(cpt):
                c = cpt * t + c4
                h_ps = hnp.tile([chunk, hidden], dt.bfloat16)
                nc.tensor.transpose(h_ps[:], ht_sb[:, c4 * chunk:(c4 + 1) * chunk],
                                    ident_sb[:])
                h_sb = hns.tile([chunk, hidden], dt.bfloat16)
                nc.vector.tensor_copy(h_sb[:], h_ps[:])
                oh = ohp.tile([chunk, gwin], dt.bfloat16)
                nc.vector.tensor_single_scalar(oh[:], iota_sb[:], bsh_sb[:, c:c + 1],
                                               mybir.AluOpType.is_equal)
                nc.tensor.matmul(s_ps[:], oh[:], h_sb[:],
                                 start=(r == 0 and c == 0),
                                 stop=(r == repeat - 1 and c == n_chunks - 1),
                                 skip_group_check=True)

        # PE is in-order, so tile t's transposes (gated on the ScalarE relu)
        # are emitted after tile t+1's main matmuls — the relu latency hides
        # under them instead of stalling the PE stream.
        pending = None
        for r in range(repeat):  # repeat>1 is a bench-only mode
            for t in range(n_tiles):
                xt = xp.tile([128, kc, tile_n], dt.float8e4)
                xsrc = xT[t, :, :].rearrange("p (c n) -> p c n", c=kc)
                ks = kc // dma_split
                for s in range(dma_split):
                    nc.sync.dma_start(xt[:, s * ks:(s + 1) * ks, :],
                                      xsrc[:, s * ks:(s + 1) * ks, :])

                ht_ps = htp.tile([hidden, tile_n], dt.float32)
                for k in range(kc // 2):
                    nc.tensor.matmul(ht_ps[:], w1_sb[:, 2 * k:2 * k + 2, :],
                                     xt[:, 2 * k:2 * k + 2, :],
                                     start=(k == 0), stop=(k == kc // 2 - 1),
                                     perf_mode=mybir.MatmulPerfMode.DoubleRow)

                if pending is not None:
                    seg_tail(*pending)

                ht_sb = hts.tile([hidden, tile_n], dt.bfloat16)
                nc.scalar.activation(ht_sb[:], ht_ps[:],
                                     mybir.ActivationFunctionType.Relu,
                                     bias=b1_sb[:], scale=1.0 / 64.0)
                pending = (r, t, ht_sb)
        seg_tail(*pending)

        s_sb = const.tile([gwin, hidden], dt.float32)
        nc.vector.tensor_copy(s_sb[:], s_ps[:])
        nc.sync.dma_start(sout[:, :], s_sb[:])

    nc.compile()
    return nc


W1_SCALE = 64.0                 # W1 pre-scale into fp8e4m3 range


def _f8dt():
    import ml_dtypes
    return np.dtype(ml_dtypes.float8_e4m3)


def _prep_w(W1, b1):
    """Weight-side input-map entries (shared by kernel() and test bench)."""
    w1_np = (np.asarray(W1, np.float32) * W1_SCALE).astype(_f8dt())
    b1_np = np.asarray(b1, np.float32).reshape(HIDDEN, 1).copy()
    return {"w1": w1_np, "b1": b1_np}


def _prep_inputs(x, batch):
    """Per-core input maps + per-core graph-window bases."""
    import ml_dtypes
    bf16 = np.dtype(ml_dtypes.bfloat16)
    f8 = _f8dt()

    iota_np = np.ascontiguousarray(
        np.broadcast_to(np.arange(GWIN, dtype=np.float32), (CHUNK, GWIN))).astype(bf16)
    ident_np = np.eye(HIDDEN, dtype=np.float32).astype(bf16)

    in_maps = []
    g_bases = []
    n_chunks = N_PAD // CHUNK
    for i in range(N_CORES):
        lo, hi = i * NPC, (i + 1) * NPC
        xs = np.zeros((N_PAD, D_FEAT), dtype=f8)
        xs[:NPC] = x[lo:hi].astype(f8)
        # tile-major pack: xTt[t, p, c*TILE_N + n] = x[t*TILE_N + n, c*128 + p]
        # so each 512-node tile is one fully-contiguous [128, 8KB] DMA.
        xT = np.ascontiguousarray(
            xs.reshape(N_TILES, TILE_N, KC, 128).transpose(0, 3, 2, 1)
        ).reshape(N_TILES, 128, KC * TILE_N)

        b = np.asarray(batch[lo:hi], dtype=np.int64)
        g0 = int(b[0])
        span = int(b[-1]) - g0
        assert span < GWIN, f"core {i}: graph span {span} >= {GWIN}"
        g_bases.append(g0)
        bshift = np.full((N_PAD,), -1.0, np.float32)
        bshift[:NPC] = (b - g0).astype(np.float32)
        bsh_np = np.ascontiguousarray(bshift.reshape(n_chunks, CHUNK).T)

        in_maps.append({
            "xT": xT,
            "bsh": bsh_np,
            "iota": iota_np,
            "ident": ident_np,
        })
    return in_maps, g_bases


def kernel(x, batch, W1, b1, W2, b2, W3, b3, W4, b4):
    global LAST_RESULT
    from concourse.bass_utils import run_bass_kernel_spmd

    x = np.asarray(x)
    batch = np.asarray(batch)

    in_maps, g_bases = _prep_inputs(x, batch)
    w_map = _prep_w(W1, b1)
    for m in in_maps:
        m.update(w_map)

    nc = _build_nc()
    res = run_bass_kernel_spmd(nc, in_maps, list(range(N_CORES)))
    LAST_RESULT = res

    # Host-side: overlap-add per-core partial segment sums, then the tiny head.
    S = np.zeros((NUM_GRAPHS + GWIN, HIDDEN), np.float64)
    for i in range(N_CORES):
        g0 = g_bases[i]
        S[g0:g0 + GWIN] += np.asarray(res.results[i]["sout"], np.float64)
    S = S[:NUM_GRAPHS]

    cnt = np.bincount(batch.astype(np.int64), minlength=NUM_GRAPHS).astype(np.float64)
    meanh = S / np.maximum(cnt, 1.0)[:, None]
    pooled = meanh @ np.asarray(W2, np.float64) + np.asarray(b2, np.float64)
    pooled *= (cnt > 0)[:, None]  # empty graphs pool to exactly zero in the reference
    z = pooled @ np.asarray(W3, np.float64) + np.asarray(b3, np.float64)
    z = z @ np.asarray(W4, np.float64) + np.asarray(b4, np.float64)
    z -= z.max(axis=0, keepdims=True)
    e = np.exp(z)
    out = e / e.sum(axis=0, keepdims=True)
    return out.astype(np.float32)



# revision 20
# speedup vs baseline: 1.2877x; 1.0467x over previous
"""Trainium2 Bass kernel for nn_Deepset GNN message-passing problem.

Computation:
    h  = relu(x @ W1 + b1)          # [N, 64]   (x: [400000, 1024])
    h2 = h @ W2 + b2                # [N, 64]
    pooled = segment_mean(h2, batch, 512)
    z = (pooled @ W3 + b3) @ W4 + b4
    out = softmax(z, axis=0)        # [512, 2]

Device does the dominant work: h = relu(x@W1+b1) and the per-graph
segment-sum of h. Everything downstream of the [512, 64] segment sums
(~2 MFLOP) runs on host (W2 commutes with the mean pool).

Sharding: data-parallel over nodes, 50000 nodes/core on 8 cores.

Device pipeline per core (fp8 DoubleRow compute, fp32 accumulation):
  - x shard is cast to fp8e4m3 and packed tile-major on host so each
    512-node tile is one fully contiguous [128 partitions x 4KB] DMA.
  - W1 is pre-scaled by 64 into fp8e4m3 range; the relu activation
    un-scales with scale=1/64 (out = relu(psum/64 + b1)).
  - PE: per 512-node tile, hT[64,512] = 4 DoubleRow matmuls, each
    contracting 256 features (lhsT [128,2,64], rhs [128,2,512]).
  - ScalarE: relu+bias (PSUM->SBUF bf16) with accum_out giving the
    per-tile row-sum T_t[64] for free.
  - Segment-sum exploits sorted `batch` + min-graph-size > tile size:
    each 512-node tile spans at most 2 consecutive graphs. DVE runs a
    prefix-sum (tensor_tensor_scan) of relu(h) along the node axis;
    GPSIMD indirect_copy gathers the cumsum column at the (data-driven)
    graph-boundary index s_t-1 => B_t[64]. Host combines:
    S[g_left] += B_t, S[g_right] += T_t - B_t.  No PE transposes, no
    one-hot matmuls: the tensor engine runs only the main matmuls.
"""

import numpy as np

N_NODES = 400000
D_FEAT = 1024
HIDDEN = 64
NUM_GRAPHS = 512
N_CORES = 8
NPC = N_NODES // N_CORES        # 50000 nodes per core
TILE_N = 512                    # nodes per PE tile
N_PAD = 50176                   # 98 * 512
N_TILES = N_PAD // TILE_N       # 98
KC = D_FEAT // 128              # 8 contraction chunks
W1_SCALE = 64.0                 # W1 pre-scale into fp8e4m3 range

LAST_RESULT = None              # BassKernelResults of the last run (for profiling)


def _build_nc(d_feat=D_FEAT, n_pad=N_PAD, tile_n=TILE_N, hidden=HIDDEN,
              repeat=1, xp_bufs=16, dma_split=2, mode="full",
              dma_engines=("sync",)):
    import concourse.bass as bass
    import concourse.bacc as bacc
    import concourse.tile as tile
    from concourse import library_config, mybir
    from contextlib import ExitStack

    dt = mybir.dt
    kc = d_feat // 128
    n_tiles = n_pad // tile_n

    nc = bacc.Bacc("TRN2", target_bir_lowering=False, debug=False)
    xT = nc.declare_dram_parameter("xT", [n_tiles, 128, kc * tile_n],
                                   dt.float8e4, isOutput=False)
    w1 = nc.declare_dram_parameter("w1", [d_feat, hidden], dt.float8e4,
                                   isOutput=False)
    b1 = nc.declare_dram_parameter("b1", [hidden, 1], dt.float32, isOutput=False)
    # per tile: col 2t = gather index (s_t - 1 at partition 16k, 511 filler
    # elsewhere), col 2t+1 = filler so each slice stays 4-byte aligned
    bidx = nc.declare_dram_parameter("bidx", [128, 2 * n_tiles], dt.int16,
                                     isOutput=False)
    # per tile, 16 gather columns: col 16t = boundary cumsum column B_t,
    # col 16t+1 = total T_t, rest filler (contiguous DMA beats compaction)
    sout = nc.declare_dram_parameter("sout", [hidden, 16 * n_tiles], dt.float32,
                                     isOutput=True)

    w1_r = w1[:, :].rearrange("(c p) h -> p c h", p=128)

    with ExitStack() as ctx:
        tc = ctx.enter_context(tile.TileContext(nc))
        const = ctx.enter_context(tc.tile_pool(name="const", bufs=1))
        xp = ctx.enter_context(tc.tile_pool(name="xp", bufs=xp_bufs))
        htp = ctx.enter_context(tc.tile_pool(name="htp", bufs=3,
                                             space=bass.MemorySpace.PSUM))
        hts = ctx.enter_context(tc.tile_pool(name="hts", bufs=3))

        nc.gpsimd.load_library(library_config.ap_gather)

        w1_sb = const.tile([128, kc, hidden], dt.float8e4)
        nc.sync.dma_start(w1_sb[:], w1_r)
        b1_sb = const.tile([hidden, 1], dt.float32)
        nc.sync.dma_start(b1_sb[:], b1[:, :])
        bidx_sb = const.tile([128, 2 * n_tiles], dt.int16)
        nc.sync.dma_start(bidx_sb[:], bidx[:, :])

        # Gather output: 16 columns per tile (0 = B_t, 1 = T_t, rest junk).
        NGI = 16
        b_sb = const.tile([128, NGI * n_tiles], dt.float32)
        # Scan buffers (manually rotated). 128 partitions because ap_gather
        # works on 128-partition data; the scan writes the lower `hidden`
        # rows, the memset keeps the junk rows finite.
        sc_tiles = [const.tile([128, tile_n], dt.float32, name=f"sc{i}")
                    for i in range(3)]
        for s in sc_tiles:
            nc.vector.memset(s[:], 0.0)

        engs = [getattr(nc, e) for e in dma_engines]
        ndma = 0
        xt0 = None
        for r in range(repeat):  # repeat>1 is a bench-only mode
            for t in range(n_tiles):
                if mode == "peonly":
                    if xt0 is None:
                        xt0 = const.tile([128, kc, tile_n], dt.float8e4)
                        xsrc = xT[0, :, :].rearrange("p (c n) -> p c n", c=kc)
                        nc.sync.dma_start(xt0[:], xsrc)
                    xt = xt0
                else:
                    xt = xp.tile([128, kc, tile_n], dt.float8e4)
                    xsrc = xT[t, :, :].rearrange("p (c n) -> p c n", c=kc)
                    ks = kc // dma_split
                    for s in range(dma_split):
                        engs[ndma % len(engs)].dma_start(
                            xt[:, s * ks:(s + 1) * ks, :],
                            xsrc[:, s * ks:(s + 1) * ks, :])
                        ndma += 1
                if mode == "dmaonly":
                    continue

                ht_ps = htp.tile([hidden, tile_n], dt.float32)
                for k in range(kc // 2):
                    nc.tensor.matmul(ht_ps[:], w1_sb[:, 2 * k:2 * k + 2, :],
                                     xt[:, 2 * k:2 * k + 2, :],
                                     start=(k == 0), stop=(k == kc // 2 - 1),
                                     perf_mode=mybir.MatmulPerfMode.DoubleRow)

                ht_sb = hts.tile([hidden, tile_n], dt.bfloat16)
                nc.scalar.activation(ht_sb[:], ht_ps[:],
                                     mybir.ActivationFunctionType.Relu,
                                     bias=b1_sb[:], scale=1.0 / W1_SCALE)

                sc = sc_tiles[t % 3]
                nc.vector.tensor_tensor_scan(sc[:hidden, :], ht_sb[:], ht_sb[:],
                                             0.0, mybir.AluOpType.add,
                                             mybir.AluOpType.bypass)

                nc.gpsimd.ap_gather(b_sb[:, NGI * t:NGI * (t + 1)], sc[:],
                                    bidx_sb[:, 2 * t:2 * t + 1],
                                    channels=128, num_elems=tile_n, d=1,
                                    num_idxs=NGI)

        if mode == "dmaonly":
            nc.any.memset(b_sb[:hidden, :], 0.0)
        nc.sync.dma_start(sout[:, :], b_sb[:hidden, :])

    nc.compile()
    return nc


def _f8dt():
    import ml_dtypes
    return np.dtype(ml_dtypes.float8_e4m3)


def _prep_w(W1, b1):
    """Weight-side input-map entries (shared by kernel() and test bench)."""
    w1_np = (np.asarray(W1, np.float32) * W1_SCALE).astype(_f8dt())
    b1_np = np.asarray(b1, np.float32).reshape(HIDDEN, 1).copy()
    return {"w1": w1_np, "b1": b1_np}


def _core_tiles(b):
    """Per-tile (g_left, split s_t, g_right, n_real) for one core's sorted
    batch slice b [NPC]."""
    out = []
    for t in range(N_TILES):
        lo = t * TILE_N
        nt = min(TILE_N, NPC - lo)
        seg = b[lo:lo + nt]
        gl = int(seg[0])
        s = int(np.searchsorted(seg, gl, side="right"))
        gr = int(seg[s]) if s < nt else -1
        if s < nt:
            # at most one boundary per tile (min graph size > TILE_N)
            assert int(seg[-1]) == gr, (
                f"tile {t}: >2 graphs in one tile ({gl}, {gr}, {int(seg[-1])})")
        out.append((gl, s, gr, nt))
    return out


def _prep_inputs(x, batch):
    """Per-core input maps + per-core tile split info for the host combine."""
    f8 = _f8dt()
    batch = np.asarray(batch, dtype=np.int64)

    in_maps = []
    tile_infos = []
    for i in range(N_CORES):
        lo, hi = i * NPC, (i + 1) * NPC
        xs = np.zeros((N_PAD, D_FEAT), dtype=f8)
        xs[:NPC] = x[lo:hi].astype(f8)
        # tile-major pack: xTt[t, p, c*TILE_N + n] = x[t*TILE_N + n, c*128 + p]
        # so each 512-node tile is one fully-contiguous [128, 4KB] DMA.
        xT = np.ascontiguousarray(
            xs.reshape(N_TILES, TILE_N, KC, 128).transpose(0, 3, 2, 1)
        ).reshape(N_TILES, 128, KC * TILE_N)

        info = _core_tiles(batch[lo:hi])
        tile_infos.append(info)
        # gather index layout: partition 16k slot 0 -> s_t-1 (B column),
        # partition 16k+1 slot 0 -> TILE_N-1 (T column), filler elsewhere.
        bidx = np.full((128, 2 * N_TILES), TILE_N - 1, np.int16)
        for t, (gl, s, gr, nt) in enumerate(info):
            bidx[0::16, 2 * t] = s - 1

        in_maps.append({"xT": xT, "bidx": bidx})
    return in_maps, tile_infos


def kernel(x, batch, W1, b1, W2, b2, W3, b3, W4, b4):
    global LAST_RESULT
    from concourse.bass_utils import run_bass_kernel_spmd

    x = np.asarray(x)
    batch = np.asarray(batch)

    in_maps, tile_infos = _prep_inputs(x, batch)
    w_map = _prep_w(W1, b1)
    for m in in_maps:
        m.update(w_map)

    nc = _build_nc()
    res = run_bass_kernel_spmd(nc, in_maps, list(range(N_CORES)))
    LAST_RESULT = res

    # Host-side: combine per-tile partial sums, then the tiny head.
    relu_b1 = np.maximum(np.asarray(b1, np.float64), 0.0)
    S = np.zeros((NUM_GRAPHS, HIDDEN), np.float64)
    for i in range(N_CORES):
        out = np.asarray(res.results[i]["sout"], np.float64)  # [64, 16*N_TILES]
        B = out[:, 0::16]
        T = out[:, 1::16]
        for t, (gl, s, gr, nt) in enumerate(tile_infos[i]):
            S[gl] += B[:, t]
            rest = T[:, t] - B[:, t]
            if nt < TILE_N:
                rest -= (TILE_N - nt) * relu_b1  # zero-padded tail nodes
            if gr >= 0:
                S[gr] += rest

    cnt = np.bincount(batch.astype(np.int64), minlength=NUM_GRAPHS).astype(np.float64)
    meanh = S / np.maximum(cnt, 1.0)[:, None]
    pooled = meanh @ np.asarray(W2, np.float64) + np.asarray(b2, np.float64)
    pooled *= (cnt > 0)[:, None]  # empty graphs pool to exactly zero in the reference
    z = pooled @ np.asarray(W3, np.float64) + np.asarray(b3, np.float64)
    z = z @ np.asarray(W4, np.float64) + np.asarray(b4, np.float64)
    z -= z.max(axis=0, keepdims=True)
    e = np.exp(z)
    out = e / e.sum(axis=0, keepdims=True)
    return out.astype(np.float32)


# revision 23
# speedup vs baseline: 1.4039x; 1.0902x over previous
"""Trainium2 Bass kernel for nn_Deepset GNN message-passing problem.

Computation:
    h  = relu(x @ W1 + b1)          # [N, 64]   (x: [400000, 1024])
    h2 = h @ W2 + b2                # [N, 64]
    pooled = segment_mean(h2, batch, 512)
    z = (pooled @ W3 + b3) @ W4 + b4
    out = softmax(z, axis=0)        # [512, 2]

Device does the dominant work: h = relu(x@W1+b1) and the per-graph
segment-sum of h. Everything downstream of the [512, 64] segment sums
(~2 MFLOP) runs on host (W2 commutes with the mean pool).

Sharding: data-parallel over nodes, 50000 nodes/core on 8 cores.

Device pipeline per core (fp8 DoubleRow compute, fp32 accumulation):
  - x shard is cast to fp8e4m3 and packed tile-major on host so each
    512-node tile is one fully contiguous [128 partitions x 4KB] DMA.
  - W1 is pre-scaled by 64 into fp8e4m3 range; the relu activation
    un-scales with scale=1/64 (out = relu(psum/64 + b1)).
  - PE: per 512-node tile, hT[64,512] = 4 DoubleRow matmuls, each
    contracting 256 features (lhsT [128,2,64], rhs [128,2,512]).
  - ScalarE: relu+bias (PSUM->SBUF bf16) with accum_out giving the
    per-tile row-sum T_t[64] for free.
  - Segment-sum exploits sorted `batch` + min-graph-size > tile size:
    each 512-node tile spans at most 2 consecutive graphs. DVE runs a
    prefix-sum (tensor_tensor_scan) of relu(h) along the node axis;
    GPSIMD indirect_copy gathers the cumsum column at the (data-driven)
    graph-boundary index s_t-1 => B_t[64]. Host combines:
    S[g_left] += B_t, S[g_right] += T_t - B_t.  No PE transposes, no
    one-hot matmuls: the tensor engine runs only the main matmuls.
"""

import numpy as np

N_NODES = 400000
D_FEAT = 1024
HIDDEN = 64
NUM_GRAPHS = 512
N_CORES = 8
NPC = N_NODES // N_CORES        # 50000 nodes per core
TILE_N = 512                    # nodes per PE tile
N_PAD = 50176                   # 98 * 512
N_TILES = N_PAD // TILE_N       # 98
KC = D_FEAT // 128              # 8 contraction chunks
W1_SCALE = 64.0                 # W1 pre-scale into fp8e4m3 range

LAST_RESULT = None              # BassKernelResults of the last run (for profiling)


def _build_nc(d_feat=D_FEAT, n_pad=N_PAD, tile_n=TILE_N, hidden=HIDDEN,
              repeat=1, xp_bufs=16, dma_split=1, mode="full",
              dma_engines=("sync",)):
    import concourse.bass as bass
    import concourse.bacc as bacc
    import concourse.tile as tile
    from concourse import library_config, mybir
    from contextlib import ExitStack

    dt = mybir.dt
    kc = d_feat // 128
    n_tiles = n_pad // tile_n

    nc = bacc.Bacc("TRN2", target_bir_lowering=False, debug=False)
    xT = nc.declare_dram_parameter("xT", [n_tiles, 128, kc * tile_n],
                                   dt.float8e4, isOutput=False)
    w1 = nc.declare_dram_parameter("w1", [d_feat, hidden], dt.float8e4,
                                   isOutput=False)
    b1 = nc.declare_dram_parameter("b1", [hidden, 1], dt.float32, isOutput=False)
    # per tile: col 2t = gather index (s_t - 1 at partition 16k, 511 filler
    # elsewhere), col 2t+1 = filler so each slice stays 4-byte aligned
    bidx = nc.declare_dram_parameter("bidx", [128, 2 * n_tiles], dt.int16,
                                     isOutput=False)
    # per tile, 16 gather columns: col 16t = boundary cumsum column B_t,
    # col 16t+1 = total T_t, rest filler (contiguous DMA beats compaction)
    sout = nc.declare_dram_parameter("sout", [hidden, 16 * n_tiles], dt.float32,
                                     isOutput=True)

    w1_r = w1[:, :].rearrange("(c p) h -> p c h", p=128)

    with ExitStack() as ctx:
        tc = ctx.enter_context(tile.TileContext(nc))
        const = ctx.enter_context(tc.tile_pool(name="const", bufs=1))
        xp = ctx.enter_context(tc.tile_pool(name="xp", bufs=xp_bufs))
        htp = ctx.enter_context(tc.tile_pool(name="htp", bufs=4,
                                             space=bass.MemorySpace.PSUM))
        hts = ctx.enter_context(tc.tile_pool(name="hts", bufs=3))

        nc.gpsimd.load_library(library_config.ap_gather)

        w1_sb = const.tile([128, kc, hidden], dt.float8e4)
        nc.sync.dma_start(w1_sb[:], w1_r)
        b1_sb = const.tile([hidden, 1], dt.float32)
        nc.sync.dma_start(b1_sb[:], b1[:, :])
        bidx_sb = const.tile([128, 2 * n_tiles], dt.int16)
        nc.sync.dma_start(bidx_sb[:], bidx[:, :])

        # Gather output: 16 columns per tile (0 = B_t, 1 = T_t, rest junk).
        NGI = 16
        b_sb = const.tile([128, NGI * n_tiles], dt.float32)
        # Scan buffers (manually rotated). 128 partitions because ap_gather
        # works on 128-partition data; the scan writes the lower `hidden`
        # rows, the memset keeps the junk rows finite.
        sc_tiles = [const.tile([128, tile_n], dt.float32, name=f"sc{i}")
                    for i in range(3)]
        for s in sc_tiles:
            nc.vector.memset(s[:], 0.0)

        engs = [getattr(nc, e) for e in dma_engines]
        ndma = 0
        xt0 = None
        assert n_tiles % 2 == 0
        for r in range(repeat):  # repeat>1 is a bench-only mode
            for tp in range(n_tiles // 2):
                pair = (2 * tp, 2 * tp + 1)
                if mode == "peonly":
                    if xt0 is None:
                        xt0 = const.tile([128, 2, kc, tile_n], dt.float8e4)
                        xsrc = xT[0:2, :, :].rearrange("t p (c n) -> p t c n",
                                                       c=kc)
                        nc.sync.dma_start(xt0[:], xsrc)
                    x2 = xt0
                else:
                    # one 1MB DMA covers both tiles of the pair
                    x2 = xp.tile([128, 2, kc, tile_n], dt.float8e4)
                    xsrc = xT[pair[0]:pair[0] + 2, :, :].rearrange(
                        "t p (c n) -> p t c n", c=kc)
                    ks = 2 // dma_split if dma_split <= 2 else 1
                    for s in range(dma_split if dma_split <= 2 else 2):
                        engs[ndma % len(engs)].dma_start(
                            x2[:, s * ks:(s + 1) * ks, :, :],
                            xsrc[:, s * ks:(s + 1) * ks, :, :])
                        ndma += 1
                if mode == "dmaonly":
                    continue

                # weight-grouped emission: both tiles' matmuls for one k-pair
                # are adjacent, so the stationary operand only changes every
                # second matmul.
                pss = []
                for j in range(2):
                    ht_ps = htp.tile([hidden, tile_n], dt.float32,
                                     name=f"ht_ps{j}")
                    pss.append(ht_ps)
                for k in range(kc // 2):
                    for j in range(2):
                        nc.tensor.matmul(pss[j][:],
                                         w1_sb[:, 2 * k:2 * k + 2, :],
                                         x2[:, j, 2 * k:2 * k + 2, :],
                                         start=(k == 0),
                                         stop=(k == kc // 2 - 1),
                                         perf_mode=mybir.MatmulPerfMode.DoubleRow)

                for j, t in enumerate(pair):
                    ht_sb = hts.tile([hidden, tile_n], dt.bfloat16)
                    nc.scalar.activation(ht_sb[:], pss[j][:],
                                         mybir.ActivationFunctionType.Relu,
                                         bias=b1_sb[:], scale=1.0 / W1_SCALE)

                    sc = sc_tiles[t % 3]
                    nc.vector.tensor_tensor_scan(sc[:hidden, :], ht_sb[:],
                                                 ht_sb[:], 0.0,
                                                 mybir.AluOpType.add,
                                                 mybir.AluOpType.bypass)

                    nc.gpsimd.ap_gather(b_sb[:, NGI * t:NGI * (t + 1)], sc[:],
                                        bidx_sb[:, 2 * t:2 * t + 1],
                                        channels=128, num_elems=tile_n, d=1,
                                        num_idxs=NGI)

        if mode == "dmaonly":
            nc.any.memset(b_sb[:hidden, :], 0.0)
        nc.sync.dma_start(sout[:, :], b_sb[:hidden, :])

    nc.compile()
    return nc


def _f8dt():
    import ml_dtypes
    return np.dtype(ml_dtypes.float8_e4m3)


def _prep_w(W1, b1):
    """Weight-side input-map entries (shared by kernel() and test bench)."""
    w1_np = (np.asarray(W1, np.float32) * W1_SCALE).astype(_f8dt())
    b1_np = np.asarray(b1, np.float32).reshape(HIDDEN, 1).copy()
    return {"w1": w1_np, "b1": b1_np}


def _core_tiles(b):
    """Per-tile (g_left, split s_t, g_right, n_real) for one core's sorted
    batch slice b [NPC]."""
    out = []
    for t in range(N_TILES):
        lo = t * TILE_N
        nt = min(TILE_N, NPC - lo)
        seg = b[lo:lo + nt]
        gl = int(seg[0])
        s = int(np.searchsorted(seg, gl, side="right"))
        gr = int(seg[s]) if s < nt else -1
        if s < nt:
            # at most one boundary per tile (min graph size > TILE_N)
            assert int(seg[-1]) == gr, (
                f"tile {t}: >2 graphs in one tile ({gl}, {gr}, {int(seg[-1])})")
        out.append((gl, s, gr, nt))
    return out


def _prep_inputs(x, batch):
    """Per-core input maps + per-core tile split info for the host combine."""
    f8 = _f8dt()
    batch = np.asarray(batch, dtype=np.int64)

    in_maps = []
    tile_infos = []
    for i in range(N_CORES):
        lo, hi = i * NPC, (i + 1) * NPC
        xs = np.zeros((N_PAD, D_FEAT), dtype=f8)
        xs[:NPC] = x[lo:hi].astype(f8)
        # tile-major pack: xTt[t, p, c*TILE_N + n] = x[t*TILE_N + n, c*128 + p]
        # so each 512-node tile is one fully-contiguous [128, 4KB] DMA.
        xT = np.ascontiguousarray(
            xs.reshape(N_TILES, TILE_N, KC, 128).transpose(0, 3, 2, 1)
        ).reshape(N_TILES, 128, KC * TILE_N)

        info = _core_tiles(batch[lo:hi])
        tile_infos.append(info)
        # gather index layout: partition 16k slot 0 -> s_t-1 (B column),
        # partition 16k+1 slot 0 -> TILE_N-1 (T column), filler elsewhere.
        bidx = np.full((128, 2 * N_TILES), TILE_N - 1, np.int16)
        for t, (gl, s, gr, nt) in enumerate(info):
            bidx[0::16, 2 * t] = s - 1

        in_maps.append({"xT": xT, "bidx": bidx})
    return in_maps, tile_infos


def kernel(x, batch, W1, b1, W2, b2, W3, b3, W4, b4):
    global LAST_RESULT
    from concourse.bass_utils import run_bass_kernel_spmd

    x = np.asarray(x)
    batch = np.asarray(batch)

    in_maps, tile_infos = _prep_inputs(x, batch)
    w_map = _prep_w(W1, b1)
    for m in in_maps:
        m.update(w_map)

    nc = _build_nc()
    res = run_bass_kernel_spmd(nc, in_maps, list(range(N_CORES)))
    LAST_RESULT = res

    # Host-side: combine per-tile partial sums, then the tiny head.
    relu_b1 = np.maximum(np.asarray(b1, np.float64), 0.0)
    S = np.zeros((NUM_GRAPHS, HIDDEN), np.float64)
    for i in range(N_CORES):
        out = np.asarray(res.results[i]["sout"], np.float64)  # [64, 16*N_TILES]
        B = out[:, 0::16]
        T = out[:, 1::16]
        for t, (gl, s, gr, nt) in enumerate(tile_infos[i]):
            S[gl] += B[:, t]
            rest = T[:, t] - B[:, t]
            if nt < TILE_N:
                rest -= (TILE_N - nt) * relu_b1  # zero-padded tail nodes
            if gr >= 0:
                S[gr] += rest

    cnt = np.bincount(batch.astype(np.int64), minlength=NUM_GRAPHS).astype(np.float64)
    meanh = S / np.maximum(cnt, 1.0)[:, None]
    pooled = meanh @ np.asarray(W2, np.float64) + np.asarray(b2, np.float64)
    pooled *= (cnt > 0)[:, None]  # empty graphs pool to exactly zero in the reference
    z = pooled @ np.asarray(W3, np.float64) + np.asarray(b3, np.float64)
    z = z @ np.asarray(W4, np.float64) + np.asarray(b4, np.float64)
    z -= z.max(axis=0, keepdims=True)
    e = np.exp(z)
    out = e / e.sum(axis=0, keepdims=True)
    return out.astype(np.float32)


# revision 28
# speedup vs baseline: 1.4928x; 1.0633x over previous
"""Trainium2 Bass kernel for nn_Deepset GNN message-passing problem.

Computation:
    h  = relu(x @ W1 + b1)          # [N, 64]   (x: [400000, 1024])
    h2 = h @ W2 + b2                # [N, 64]
    pooled = segment_mean(h2, batch, 512)
    z = (pooled @ W3 + b3) @ W4 + b4
    out = softmax(z, axis=0)        # [512, 2]

Device does the dominant work: h = relu(x@W1+b1) and the per-graph
segment-sum of h. Everything downstream of the [512, 64] segment sums
(~2 MFLOP) runs on host (W2 commutes with the mean pool).

Sharding: data-parallel over nodes, 50000 nodes/core on 8 cores.

Device pipeline per core (fp8 DoubleRow compute, fp32 accumulation):
  - x shard is cast to fp8e4m3 and packed tile-major on host so each
    512-node tile is one fully contiguous [128 partitions x 4KB] DMA.
  - W1 is pre-scaled by 64 into fp8e4m3 range; the relu activation
    un-scales with scale=1/64 (out = relu(psum/64 + b1)).
  - PE: per 512-node tile, hT[64,512] = 4 DoubleRow matmuls, each
    contracting 256 features (lhsT [128,2,64], rhs [128,2,512]).
  - ScalarE: relu+bias (PSUM->SBUF bf16) with accum_out giving the
    per-tile row-sum T_t[64] for free.
  - Segment-sum exploits sorted `batch` + min-graph-size > tile size:
    each 512-node tile spans at most 2 consecutive graphs. DVE runs a
    prefix-sum (tensor_tensor_scan) of relu(h) along the node axis;
    GPSIMD indirect_copy gathers the cumsum column at the (data-driven)
    graph-boundary index s_t-1 => B_t[64]. Host combines:
    S[g_left] += B_t, S[g_right] += T_t - B_t.  No PE transposes, no
    one-hot matmuls: the tensor engine runs only the main matmuls.
"""

import numpy as np

N_NODES = 400000
D_FEAT = 1024
HIDDEN = 64
NUM_GRAPHS = 512
N_CORES = 8
NPC = N_NODES // N_CORES        # 50000 nodes per core
TILE_N = 512                    # nodes per PE tile
N_PAD = 50176                   # 98 * 512
N_TILES = N_PAD // TILE_N       # 98
KC = D_FEAT // 128              # 8 contraction chunks
W1_SCALE = 64.0                 # W1 pre-scale into fp8e4m3 range

LAST_RESULT = None              # BassKernelResults of the last run (for profiling)


def _build_nc(d_feat=D_FEAT, n_pad=N_PAD, tile_n=TILE_N, hidden=HIDDEN,
              repeat=1, xp_bufs=16, dma_split=1, mode="full",
              dma_engines=("sync",)):
    import concourse.bass as bass
    import concourse.bacc as bacc
    import concourse.tile as tile
    from concourse import library_config, mybir
    from contextlib import ExitStack

    dt = mybir.dt
    kc = d_feat // 128
    n_tiles = n_pad // tile_n

    nc = bacc.Bacc("TRN2", target_bir_lowering=False, debug=False)
    xT = nc.declare_dram_parameter("xT", [n_tiles, 128, kc * tile_n],
                                   dt.float8e4, isOutput=False)
    # SwInterleave layout: per k-pair, [128, 2*hidden] with (A,B) interleaved
    # per column and hidden columns reversed (see _prep_w).
    w1 = nc.declare_dram_parameter("w1", [kc // 2, 128, 2 * hidden],
                                   dt.float8e4, isOutput=False)
    b1 = nc.declare_dram_parameter("b1", [hidden, 1], dt.float32, isOutput=False)
    # per tile: col 2t = gather index (s_t - 1 at partition 16k, 511 filler
    # elsewhere), col 2t+1 = filler so each slice stays 4-byte aligned
    bidx = nc.declare_dram_parameter("bidx", [128, 2 * n_tiles], dt.int16,
                                     isOutput=False)
    # per tile, 16 gather columns: col 16t = boundary cumsum column B_t,
    # col 16t+1 = total T_t, rest filler (contiguous DMA beats compaction)
    sout = nc.declare_dram_parameter("sout", [hidden, 16 * n_tiles], dt.float32,
                                     isOutput=True)

    w1_r = w1[:, :, :].rearrange("c p h -> p c h")

    with ExitStack() as ctx:
        tc = ctx.enter_context(tile.TileContext(nc))
        const = ctx.enter_context(tc.tile_pool(name="const", bufs=1))
        xp = ctx.enter_context(tc.tile_pool(name="xp", bufs=xp_bufs))
        htp = ctx.enter_context(tc.tile_pool(name="htp", bufs=4,
                                             space=bass.MemorySpace.PSUM))
        hts = ctx.enter_context(tc.tile_pool(name="hts", bufs=3))

        nc.gpsimd.load_library(library_config.ap_gather)

        w1_sb = const.tile([128, kc // 2, 2 * hidden], dt.float8e4)
        nc.sync.dma_start(w1_sb[:], w1_r)
        b1_sb = const.tile([hidden, 1], dt.float32)
        nc.sync.dma_start(b1_sb[:], b1[:, :])
        bidx_sb = const.tile([128, 2 * n_tiles], dt.int16)
        nc.sync.dma_start(bidx_sb[:], bidx[:, :])

        # Gather output: 16 columns per tile (0 = B_t, 1 = T_t, rest junk).
        NGI = 16
        b_sb = const.tile([128, NGI * n_tiles], dt.float32)
        # Scan buffers (manually rotated). 128 partitions because ap_gather
        # works on 128-partition data; the scan writes the lower `hidden`
        # rows, the memset keeps the junk rows finite.
        sc_tiles = [const.tile([128, tile_n], dt.float32, name=f"sc{i}")
                    for i in range(3)]
        for s in sc_tiles:
            nc.vector.memset(s[:], 0.0)

        engs = [getattr(nc, e) for e in dma_engines]
        ndma = 0
        xt0 = None
        assert n_tiles % 2 == 0
        for r in range(repeat):  # repeat>1 is a bench-only mode
            for tp in range(n_tiles // 2):
                pair = (2 * tp, 2 * tp + 1)
                if mode == "peonly":
                    if xt0 is None:
                        xt0 = const.tile([128, 2, kc, tile_n], dt.float8e4)
                        xsrc = xT[0:2, :, :].rearrange("t p (c n) -> p t c n",
                                                       c=kc)
                        nc.sync.dma_start(xt0[:], xsrc)
                    x2 = xt0
                else:
                    # one 1MB DMA covers both tiles of the pair
                    x2 = xp.tile([128, 2, kc, tile_n], dt.float8e4)
                    xsrc = xT[pair[0]:pair[0] + 2, :, :].rearrange(
                        "t p (c n) -> p t c n", c=kc)
                    ks = 2 // dma_split if dma_split <= 2 else 1
                    for s in range(dma_split if dma_split <= 2 else 2):
                        engs[ndma % len(engs)].dma_start(
                            x2[:, s * ks:(s + 1) * ks, :, :],
                            xsrc[:, s * ks:(s + 1) * ks, :, :])
                        ndma += 1
                if mode == "dmaonly":
                    continue

                # weight-grouped emission: both tiles' matmuls for one k-pair
                # are adjacent, so the stationary operand only changes every
                # second matmul.
                pss = []
                for j in range(2):
                    ht_ps = htp.tile([hidden, tile_n], dt.float32,
                                     name=f"ht_ps{j}")
                    pss.append(ht_ps)
                for k in range(kc // 2):
                    for j in range(2):
                        nc.tensor.matmul(
                            pss[j][:], w1_sb[:, k, :],
                            x2[:, j, 2 * k:2 * k + 2, :],
                            start=(k == 0), stop=(k == kc // 2 - 1),
                            perf_mode=mybir.MatmulPerfMode.DoubleRowSwInterleave)

                for j, t in enumerate(pair):
                    ht_sb = hts.tile([hidden, tile_n], dt.bfloat16)
                    nc.scalar.activation(ht_sb[:], pss[j][:],
                                         mybir.ActivationFunctionType.Relu,
                                         bias=b1_sb[:], scale=1.0 / W1_SCALE)

                    sc = sc_tiles[t % 3]
                    nc.vector.tensor_tensor_scan(sc[:hidden, :], ht_sb[:],
                                                 ht_sb[:], 0.0,
                                                 mybir.AluOpType.add,
                                                 mybir.AluOpType.bypass)

                    nc.gpsimd.ap_gather(b_sb[:, NGI * t:NGI * (t + 1)], sc[:],
                                        bidx_sb[:, 2 * t:2 * t + 1],
                                        channels=128, num_elems=tile_n, d=1,
                                        num_idxs=NGI)

        if mode == "dmaonly":
            nc.any.memset(b_sb[:hidden, :], 0.0)
        nc.sync.dma_start(sout[:, :], b_sb[:hidden, :])

    nc.compile()
    return nc


def _f8dt():
    import ml_dtypes
    return np.dtype(ml_dtypes.float8_e4m3)


def _prep_w(W1, b1):
    """Weight-side input-map entries (shared by kernel() and test bench).

    DoubleRowSwInterleave layout: per 256-row k-pair, [128, 2*HIDDEN] where
    column 2u+i holds W[pair*256 + i*128 + p, HIDDEN-1-u] (pairs interleaved
    per column, hidden columns reversed)."""
    w = (np.asarray(W1, np.float32) * W1_SCALE).astype(_f8dt())
    wr = w.reshape(KC // 2, 2, 128, HIDDEN)          # [pair, i, p, m]
    w_sw = wr[:, :, :, ::-1].transpose(0, 2, 3, 1)   # [pair, p, u, i]
    w1_np = np.ascontiguousarray(w_sw).reshape(KC // 2, 128, 2 * HIDDEN)
    b1_np = np.asarray(b1, np.float32).reshape(HIDDEN, 1).copy()
    return {"w1": w1_np, "b1": b1_np}


def _core_tiles(b):
    """Per-tile (g_left, split s_t, g_right, n_real) for one core's sorted
    batch slice b [NPC]."""
    out = []
    for t in range(N_TILES):
        lo = t * TILE_N
        nt = min(TILE_N, NPC - lo)
        seg = b[lo:lo + nt]
        gl = int(seg[0])
        s = int(np.searchsorted(seg, gl, side="right"))
        gr = int(seg[s]) if s < nt else -1
        if s < nt:
            # at most one boundary per tile (min graph size > TILE_N)
            assert int(seg[-1]) == gr, (
                f"tile {t}: >2 graphs in one tile ({gl}, {gr}, {int(seg[-1])})")
        out.append((gl, s, gr, nt))
    return out


def _prep_inputs(x, batch):
    """Per-core input maps + per-core tile split info for the host combine."""
    f8 = _f8dt()
    batch = np.asarray(batch, dtype=np.int64)

    in_maps = []
    tile_infos = []
    for i in range(N_CORES):
        lo, hi = i * NPC, (i + 1) * NPC
        xs = np.zeros((N_PAD, D_FEAT), dtype=f8)
        xs[:NPC] = x[lo:hi].astype(f8)
        # tile-major pack: xTt[t, p, c*TILE_N + n] = x[t*TILE_N + n, c*128 + p]
        # so each 512-node tile is one fully-contiguous [128, 4KB] DMA.
        xT = np.ascontiguousarray(
            xs.reshape(N_TILES, TILE_N, KC, 128).transpose(0, 3, 2, 1)
        ).reshape(N_TILES, 128, KC * TILE_N)

        info = _core_tiles(batch[lo:hi])
        tile_infos.append(info)
        # gather index layout: partition 16k slot 0 -> s_t-1 (B column),
        # partition 16k+1 slot 0 -> TILE_N-1 (T column), filler elsewhere.
        bidx = np.full((128, 2 * N_TILES), TILE_N - 1, np.int16)
        for t, (gl, s, gr, nt) in enumerate(info):
            bidx[0::16, 2 * t] = s - 1

        in_maps.append({"xT": xT, "bidx": bidx})
    return in_maps, tile_infos


def kernel(x, batch, W1, b1, W2, b2, W3, b3, W4, b4):
    global LAST_RESULT
    from concourse.bass_utils import run_bass_kernel_spmd

    x = np.asarray(x)
    batch = np.asarray(batch)

    in_maps, tile_infos = _prep_inputs(x, batch)
    w_map = _prep_w(W1, b1)
    for m in in_maps:
        m.update(w_map)

    nc = _build_nc()
    res = run_bass_kernel_spmd(nc, in_maps, list(range(N_CORES)))
    LAST_RESULT = res

    # Host-side: combine per-tile partial sums, then the tiny head.
    relu_b1 = np.maximum(np.asarray(b1, np.float64), 0.0)
    S = np.zeros((NUM_GRAPHS, HIDDEN), np.float64)
    for i in range(N_CORES):
        out = np.asarray(res.results[i]["sout"], np.float64)  # [64, 16*N_TILES]
        B = out[:, 0::16]
        T = out[:, 1::16]
        for t, (gl, s, gr, nt) in enumerate(tile_infos[i]):
            S[gl] += B[:, t]
            rest = T[:, t] - B[:, t]
            if nt < TILE_N:
                rest -= (TILE_N - nt) * relu_b1  # zero-padded tail nodes
            if gr >= 0:
                S[gr] += rest

    cnt = np.bincount(batch.astype(np.int64), minlength=NUM_GRAPHS).astype(np.float64)
    meanh = S / np.maximum(cnt, 1.0)[:, None]
    pooled = meanh @ np.asarray(W2, np.float64) + np.asarray(b2, np.float64)
    pooled *= (cnt > 0)[:, None]  # empty graphs pool to exactly zero in the reference
    z = pooled @ np.asarray(W3, np.float64) + np.asarray(b3, np.float64)
    z = z @ np.asarray(W4, np.float64) + np.asarray(b4, np.float64)
    z -= z.max(axis=0, keepdims=True)
    e = np.exp(z)
    out = e / e.sum(axis=0, keepdims=True)
    return out.astype(np.float32)
